# revision 1
# baseline (speedup 1.0000x reference)
"""Distributed CharRNN LSTM kernel for 8 Trainium2 NeuronCores (Bass/Tile).

Strategy: shard the hidden dimension H=2048 across the 8 cores (256 h-rows and
the matching 1024 gate rows each; gate column order [f, o, i, g]). Weights for
the recurrent GEMM live in SBUF in bf16. Each step computes
gates = xproj[t] + h_full @ w_hhT_k as 16 K-chunk matmuls (fp32 PSUM), applies
a tanh-only gate nonlinearity (sigmoid(x) = 0.5*tanh(x/2)+0.5 with the 0.5
pre-scales folded into host-prepared weights), updates doubled states
(Sc=2c, Sh=2h), computes exp(h)+row-sum partials for the per-step softmax, and
AllGathers the transposed h slices (+partials) so every core has the full h for
the next step. The input projection GEMM (input @ w_ihT + biases) runs on
device, interleaved with the scan in an SBUF ring. Softmax denominators come
one step later from the gathered partials (exp needs no max subtraction since
h is in (-1,1)).

kernel(**inputs) takes the full unsharded numpy inputs and returns
(ys [T,B,H], h_T [B,H], c_T [B,H]) matching the reference.
"""
import sys
sys.path.insert(0, "/opt/trn_rl_repo")
import numpy as np

import concourse.bass as bass
import concourse.bacc as bacc
import concourse.mybir as mybir
import concourse.tile as tile
from concourse.bass_utils import run_bass_kernel_spmd

BF16 = mybir.dt.bfloat16
F32 = mybir.dt.float32
NP_BF16 = mybir.dt.np(BF16)

T, B, X, H = 256, 64, 512, 2048
NCORES = 8
HS = H // NCORES            # 256 h rows per core
GS = 4 * HS                 # 1024 gate rows per core
PAY = 144                   # exchange payload width (0:128 data, 128 partial)
AF = mybir.ActivationFunctionType
ALU = mybir.AluOpType


def _host_prep(inputs, perms):
    inp = np.asarray(inputs["input"], np.float32)
    h_0 = np.asarray(inputs["h_0"], np.float32)
    c_0 = np.asarray(inputs["c_0"], np.float32)
    w_ih = np.asarray(inputs["w_ih"], np.float32)
    w_hh = np.asarray(inputs["w_hh"], np.float32)
    xb = (np.asarray(inputs["b_ih"], np.float32)
          + np.asarray(inputs["b_hh"], np.float32))

    Tn = inp.shape[0]
    A = inp.reshape(Tn * B, X).T
    n_tiles = (Tn * B) // 128
    inpT_tiled = np.ascontiguousarray(
        A.reshape(4, 128, n_tiles, 128).transpose(2, 1, 0, 3)).astype(NP_BF16)

    identB = np.ascontiguousarray(
        np.tile(np.eye(64, dtype=np.float32), (2, 1))).astype(NP_BF16)
    identF = np.ascontiguousarray(
        np.tile(np.eye(64, dtype=np.float32), (2, 1)))

    maps = []
    for k in range(NCORES):
        rows = np.concatenate([
            np.arange(g * H + k * HS, g * H + (k + 1) * HS)
            for g in (1, 3, 0, 2)])                    # [f, o, i, g]
        scale = np.concatenate([
            np.full(HS, 0.5, np.float32),
            np.full(HS, 0.5, np.float32),
            np.full(HS, 0.5, np.float32),
            np.ones(HS, np.float32)])
        whhT = (w_hh[rows, :].T * scale) * 0.5         # x0.5: doubled Sh state
        perm = perms[k]
        row_order = np.concatenate([
            np.arange(perm[j] * HS, (perm[j] + 1) * HS) for j in range(NCORES)])
        whhT = np.ascontiguousarray(whhT[row_order])
        whh_sb = np.ascontiguousarray(
            whhT.reshape(16, 128, GS).transpose(1, 0, 2)).astype(NP_BF16)
        wihT = np.ascontiguousarray((w_ih[rows, :].T * scale))
        wih_sb = np.ascontiguousarray(
            wihT.reshape(4, 128, GS).transpose(1, 0, 2)).astype(NP_BF16)
        xb_k = np.ascontiguousarray((xb[rows] * scale)[None, :]
                                    .repeat(128, 0)).astype(np.float32)
        h0g = np.zeros((128, NCORES, PAY), np.float32)
        for j in range(NCORES):
            s = perm[j]
            blk = 2.0 * h_0[:, s * HS:(s + 1) * HS].T
            h0g[:, j, 0:64] = blk[0:128, :]
            h0g[:, j, 64:128] = blk[128:256, :]
        maps.append({
            "inpT": inpT_tiled,
            "whh": whh_sb,
            "wih": wih_sb,
            "xbB": xb_k,
            "identB": identB,
            "identF": identF,
            "h0g": h0g.astype(NP_BF16),
            "c0": np.ascontiguousarray(2.0 * c_0[:, k * HS:(k + 1) * HS]),
        })
    return maps


def _build(n_steps, xp_lookahead=4):
    nc = bacc.Bacc("TRN2", target_bir_lowering=False, debug=False,
                   num_devices=NCORES)
    n_tiles = (n_steps * B) // 128

    inpT = nc.declare_dram_parameter("inpT", [n_tiles, 128, 4, 128], BF16, isOutput=False)
    whh_d = nc.declare_dram_parameter("whh", [128, 16, GS], BF16, isOutput=False)
    wih_d = nc.declare_dram_parameter("wih", [128, 4, GS], BF16, isOutput=False)
    xbB_d = nc.declare_dram_parameter("xbB", [128, GS], F32, isOutput=False)
    idB_d = nc.declare_dram_parameter("identB", [128, 64], BF16, isOutput=False)
    idF_d = nc.declare_dram_parameter("identF", [128, 64], F32, isOutput=False)
    h0g_d = nc.declare_dram_parameter("h0g", [128, NCORES, PAY], BF16, isOutput=False)
    c0_d = nc.declare_dram_parameter("c0", [64, HS], F32, isOutput=False)
    y_d = nc.declare_dram_parameter("y_out", [n_steps, 64, HS], F32, isOutput=True)
    h_d = nc.declare_dram_parameter("h_out", [64, HS], F32, isOutput=True)
    c_d = nc.declare_dram_parameter("c_out", [64, HS], F32, isOutput=True)

    with tile.TileContext(nc) as tc:
        with (
            tc.tile_pool(name="const", bufs=1) as cpool,
            tc.tile_pool(name="xpr", bufs=8) as xpr,
            tc.tile_pool(name="work", bufs=2) as work,
            tc.tile_pool(name="ps_g", bufs=2, space="PSUM") as ps_g,
            tc.tile_pool(name="ps_x", bufs=1, space="PSUM") as ps_x,
            tc.tile_pool(name="ps_t", bufs=2, space="PSUM") as ps_t,
            tc.tile_pool(name="dram", bufs=2, space="DRAM") as dram,
        ):
            whh = cpool.tile([128, 16, GS], BF16, tag="whh")
            wih = cpool.tile([128, 4, GS], BF16, tag="wih")
            xbB = cpool.tile([128, GS], F32, tag="xbB")
            identB = cpool.tile([128, 64], BF16, tag="identB")
            identF = cpool.tile([128, 64], F32, tag="identF")
            nc.sync.dma_start(whh[:], whh_d[:])
            nc.sync.dma_start(wih[:], wih_d[:])
            nc.sync.dma_start(xbB[:], xbB_d[:])
            nc.sync.dma_start(identB[:], idB_d[:])
            nc.sync.dma_start(identF[:], idF_d[:])

            c_prev = work.tile([64, HS], F32, tag="c")
            nc.sync.dma_start(c_prev[:], c0_d[:])
            gbuf = work.tile([128, NCORES, PAY], BF16, tag="gbuf")
            nc.sync.dma_start(gbuf[:], h0g_d[:])

            xp_tiles = []

            def emit_xproj_tile(m):
                xin = work.tile([128, 4, 128], BF16, tag="xin")
                nc.sync.dma_start(xin[:], inpT[m][:])
                xa = ps_x.tile([128, GS], F32, tag="xacc")
                for x in range(4):
                    nc.tensor.matmul(xa[:, 0:512], xin[:, x, :], wih[:, x, 0:512],
                                     start=(x == 0), stop=(x == 3))
                    nc.tensor.matmul(xa[:, 512:GS], xin[:, x, :], wih[:, x, 512:GS],
                                     start=(x == 0), stop=(x == 3))
                xp = xpr.tile([128, GS], BF16, tag="xp")
                nc.vector.tensor_add(xp[0:64, :], xa[0:64, :], xbB[0:64, :])
                nc.vector.tensor_add(xp[64:128, :], xa[64:128, :], xbB[64:128, :])
                xp_tiles.append(xp)

            for m in range(min(xp_lookahead, n_tiles)):
                emit_xproj_tile(m)

            e_prev, hn_prev, cn_prev = None, None, None
            gbuf_prev = None

            for t in range(n_steps):
                if t % 2 == 0:
                    m = t // 2 + xp_lookahead
                    if m < n_tiles:
                        emit_xproj_tile(m)
                xp = xp_tiles[t // 2]
                xp_half = xp[0:64, :] if t % 2 == 0 else xp[64:128, :]
                iid = identB[0:64, :] if t % 2 == 0 else identB[64:128, :]

                gp = ps_g.tile([64, GS], F32, tag="gates")
                nc.tensor.matmul(gp[:, 0:512], iid, xp_half[:, 0:512],
                                 start=True, stop=False)
                nc.tensor.matmul(gp[:, 512:GS], iid, xp_half[:, 512:GS],
                                 start=True, stop=False)
                for jc in range(16):
                    j, cc = jc // 2, jc % 2
                    lhsT = gbuf[:, j, cc * 64:(cc + 1) * 64]
                    nc.tensor.matmul(gp[:, 0:512], lhsT, whh[:, jc, 0:512],
                                     start=False, stop=(jc == 15))
                    nc.tensor.matmul(gp[:, 512:GS], lhsT, whh[:, jc, 512:GS],
                                     start=False, stop=(jc == 15))

                gt = work.tile([64, GS], F32, tag="gt")
                nc.scalar.activation(gt[:], gp[:], AF.Tanh)
                m1 = work.tile([64, HS], F32, tag="m1")
                nc.vector.scalar_tensor_tensor(m1[:], gt[:, 0:256], 1.0,
                                               c_prev[:], ALU.add, ALU.mult)
                m2 = work.tile([64, HS], F32, tag="m2")
                nc.vector.scalar_tensor_tensor(m2[:], gt[:, 512:768], 1.0,
                                               gt[:, 768:GS], ALU.add, ALU.mult)
                c_new = work.tile([64, HS], F32, tag="c")
                nc.vector.scalar_tensor_tensor(c_new[:], m1[:], 0.5,
                                               m2[:], ALU.mult, ALU.add)
                tc_t = work.tile([64, HS], F32, tag="tanh_c")
                nc.scalar.activation(tc_t[:], c_new[:], AF.Tanh, scale=0.5)
                h_new = work.tile([64, HS], F32, tag="h_new")
                nc.vector.scalar_tensor_tensor(h_new[:], gt[:, 256:512], 1.0,
                                               tc_t[:], ALU.add, ALU.mult)

                e_own = work.tile([64, HS], F32, tag="e_own")
                partial = work.tile([64, 1], F32, tag="partial")
                nc.scalar.activation(e_own[:], h_new[:], AF.Exp, scale=0.5,
                                     accum_out=partial[:])

                pt = ps_t.tile([128, 128], F32, tag="pt")
                nc.tensor.transpose(pt[:, 0:64], h_new[:, 0:128], identF[0:64, :])
                nc.tensor.transpose(pt[:, 64:128], h_new[:, 128:256], identF[0:64, :])
                payload = work.tile([128, PAY], BF16, tag="payload")
                nc.vector.tensor_copy(payload[:, 0:128], pt[:])
                nc.vector.tensor_copy(payload[0:64, 128:129], partial[:])

                in_b = dram.tile([128, PAY], BF16, tag="in_b")
                out_b = dram.tile([NCORES * 128, PAY], BF16, tag="out_b")
                nc.sync.dma_start(in_b[:], payload[:])
                nc.gpsimd.collective_compute(
                    "AllGather", ALU.bypass,
                    replica_groups=[list(range(NCORES))],
                    ins=[in_b.opt()], outs=[out_b.opt()])
                gbuf = work.tile([128, NCORES, PAY], BF16, tag="gbuf")
                nc.sync.dma_start(
                    gbuf[:], out_b[:].rearrange("(r p) f -> p r f", p=128))

                if e_prev is not None:
                    denom = work.tile([64, 1], F32, tag="denom")
                    nc.vector.reduce_sum(denom[:], gbuf_prev[0:64, :, 128],
                                         axis=mybir.AxisListType.X)
                    rcp = work.tile([64, 1], F32, tag="rcp")
                    nc.vector.reciprocal(rcp[:], denom[:])
                    y_sb = work.tile([64, HS], F32, tag="y_sb")
                    nc.vector.tensor_scalar(y_sb[:], e_prev[:], rcp[:], None,
                                            ALU.mult)
                    nc.sync.dma_start(y_d[t - 1][:], y_sb[:])

                gbuf_prev = gbuf
                e_prev, hn_prev, cn_prev = e_own, h_new, c_new
                c_prev = c_new

            denom = work.tile([64, 1], F32, tag="denom")
            nc.vector.reduce_sum(denom[:], gbuf[0:64, :, 128],
                                 axis=mybir.AxisListType.X)
            rcp = work.tile([64, 1], F32, tag="rcp")
            nc.vector.reciprocal(rcp[:], denom[:])
            y_sb = work.tile([64, HS], F32, tag="y_sb")
            nc.vector.tensor_scalar(y_sb[:], e_prev[:], rcp[:], None, ALU.mult)
            nc.sync.dma_start(y_d[n_steps - 1][:], y_sb[:])
            h_fin = work.tile([64, HS], F32, tag="h_fin")
            nc.vector.tensor_scalar_mul(h_fin[:], hn_prev[:], 0.5)
            nc.sync.dma_start(h_d[:], h_fin[:])
            c_fin = work.tile([64, HS], F32, tag="c_fin")
            nc.vector.tensor_scalar_mul(c_fin[:], cn_prev[:], 0.5)
            nc.sync.dma_start(c_d[:], c_fin[:])

    nc.compile()
    return nc


_NC_CACHE = {}


def _get_nc(n_steps=T):
    if n_steps not in _NC_CACHE:
        _NC_CACHE[n_steps] = _build(n_steps)
    return _NC_CACHE[n_steps]


def kernel(**inputs):
    n_steps = int(np.asarray(inputs["input"]).shape[0])
    nc = _get_nc(n_steps)
    perms = [list(range(NCORES))] * NCORES
    maps = _host_prep(inputs, perms)
    res = run_bass_kernel_spmd(nc, maps, list(range(NCORES))).results
    ys = np.concatenate([r["y_out"] for r in res], axis=-1)
    h_T = np.concatenate([r["h_out"] for r in res], axis=-1)
    c_T = np.concatenate([r["c_out"] for r in res], axis=-1)
    return ys, h_T, c_T
